# revision 91
# baseline (speedup 1.0000x reference)
"""Trainium2 Bass kernel for nn_Attention_block (retrieval_knn).

Reference (per sample b, match A in {Q_flo, K_dep}, V = V_rgb):
  T[i,j] = <A[:,i], V[:,j]>          [4096, 4096] score matrix
  S[j] = max_i T ; idx[j] = argmax_i T
  C = conv1x1([V; A[:, idx]]) * S    (conv1: 128->64)
  fused = [C_v, C_k, V]              (192 ch)
  y = relu(BN(conv3x3(fused)))       (conv2: 192->64, pad 1)

Sharding: 8 cores = 4 samples x 2 W-halves (pure data parallel; each core
takes a 1-row halo each side of its half for the 3x3 conv and computes its
2176 j-columns against the full 4096-long i axis).

Key speed structure vs the fp32 baseline:
  - Scores are computed as an fp16 3-term split (T = <Ah,Vh> + <Ah,Vl>
    + <Al,Vh>, terms accumulated in the same PSUM bank): 3 PE cycles/row
    instead of fp32's 4, with max score error ~8e-6 -- 18x below the
    smallest top-2 gap on this input (1.43e-4), so no argmax flips.
    (float32r at 1 cyc/row was measured at ~6e-3 score error: flips.)
  - All other matmuls (G', conv1, conv2, transposes) run pure fp16
    (1 cyc/row; output tolerance is 2e-2, fp16 path lands ~1e-3).
  - PSUM evacuation by ACT in [128,1024] 2-bank units (fewer fixed costs).
  - conv1 runs in j-major layout: the argmax gather already yields
    gtile[j, c], the 1x1 V-part matmul outputs [j, c] directly, the *S
    scale is a free ACT per-partition scale (S lives as [j, 1]), and a PE
    transpose returns [c, j] for conv2 -- no DVE multiplies, no S
    broadcast round trip through DRAM.
  - G-trick: gather commutes with conv1's TA half:
      conv1([V;TA]) + b1 = W1v@V + (W1t@A + b1)[:, idx]
    G' = W1t@A + b1 computed once, transposed to DRAM [4096, 64] fp16,
    argmax rows fetched by indirect-DMA gather.
  - Argmax spine per 128-j tile: DVE does one 2x-mode max-accum pass over
    the evacuated fp32 T row [128, 4096] and one 1x is_equal*iota pass
    (sum-accum -> index; max is unique per row on this input).
"""

import numpy as np

import concourse.bass as bass
import concourse.bacc as bacc
import concourse.mybir as mybir
from concourse.tile import TileContext
from concourse import bass_utils

F32 = mybir.dt.float32
F16 = mybir.dt.float16
I32 = mybir.dt.int32
AF = mybir.ActivationFunctionType
OP = mybir.AluOpType

B, C, W, H = 4, 64, 64, 64
HW = W * H                     # 4096
BN_EPS = 1e-5
N_CORES = 8
NROWS = W // 2 + 1             # 33 stored rows: 32 own + 1 interior halo
NSLOT = NROWS + 1              # 34 fused slots (slot 0 = zero row-pad)
JW = NROWS * H                 # 2112 j-columns per core
JT = 17                        # 16 interior j-tiles + 1 mixed q/k halo tile
NCH = HW // 512                # 8 i-chunks
OUT_ROWS = W // 2              # 32 interior rows per core
YPAD = H + 2                   # 66 padded y positions in fused layout

NEG = -3.0e38


def _build_nc():
    nc = bacc.Bacc("TRN2", target_bir_lowering=False)

    a_hi = nc.dram_tensor("a_hi", [128, HW], F16, kind="ExternalInput")
    a_lo = nc.dram_tensor("a_lo", [128, HW], F16, kind="ExternalInput")
    v_hi = nc.dram_tensor("v_hi", [128, JW], F16, kind="ExternalInput")
    v_lo = nc.dram_tensor("v_lo", [128, JW], F16, kind="ExternalInput")
    w1vt = nc.dram_tensor("w1vt", [128, C], F16, kind="ExternalInput")
    w1tt = nc.dram_tensor("w1tt", [128, C], F16, kind="ExternalInput")
    b1d = nc.dram_tensor("b1d", [C, 1], F32, kind="ExternalInput")
    w2ad = nc.dram_tensor("w2ad", [128, 9 * C], F16, kind="ExternalInput")
    w2bd = nc.dram_tensor("w2bd", [C, 9 * C], F16, kind="ExternalInput")
    bnad = nc.dram_tensor("bnad", [128, 1], F32, kind="ExternalInput")
    bnbd = nc.dram_tensor("bnbd", [128, 1], F32, kind="ExternalInput")
    yout = nc.dram_tensor("y", [C, OUT_ROWS * H], F32, kind="ExternalOutput")

    iota_d = nc.inline_tensor(
        np.broadcast_to(np.arange(HW, dtype=np.float32), (128, HW)).copy(),
        name="iota4096")
    ident_d = nc.inline_tensor(np.eye(128, dtype=np.float16), name="ident16")

    with TileContext(nc) as tc:
        with tc.tile_pool(name="persist", bufs=1) as pp:
            ah_t = pp.tile([128, HW], F16)
            al_t = pp.tile([128, HW], F16)
            vh_t = pp.tile([128, JW], F16)
            vl_t = pp.tile([128, JW], F16)
            w1vt_t = pp.tile([128, C], F16)
            w1tt_t = pp.tile([128, C], F16)
            b1_t = pp.tile([C, 1], F32)
            w2a_t = pp.tile([128, 9 * C], F16)
            w2b_t = pp.tile([C, 9 * C], F16)
            bna_t = pp.tile([128, 1], F32)
            bnb_t = pp.tile([128, 1], F32)
            iota_t = pp.tile([128, HW], F32)
            ident = pp.tile([128, 128], F16)
            gtile = [pp.tile([128, JT * C], F16, tag="gtq", name="gtq_t"),
                     pp.tile([128, JT * C], F16, tag="gtk", name="gtk_t")]
            gth = [pp.tile([128, C], F16, tag="gthq", name="gthq_t"),
                   pp.tile([128, C], F16, tag="gthk", name="gthk_t")]
            s_all = [pp.tile([128, JT], F32, tag="sq", name="sq_t"),
                     pp.tile([128, JT], F32, tag="sk", name="sk_t")]
            idx_all = [pp.tile([128, JT], I32, tag="idxq", name="idxq_t"),
                       pp.tile([128, JT], I32, tag="idxk", name="idxk_t")]
            fused_a = pp.tile([128, NSLOT * YPAD], F16)   # C_v / C_k
            fused_b = pp.tile([C, NSLOT * YPAD], F16)     # V, y-padded
            out_sb = pp.tile([128, OUT_ROWS * H // 2], F32)

            # --- loads, spine inputs first so the first score matmuls can
            # start ASAP.  A tensors stacked [128, HW] (q rows 0-63, k rows
            # 64-127); V duplicated onto both partition halves so the q/k
            # matmuls can address their own partition range.
            nc.sync.dma_start(out=vh_t[:, 0:128], in_=v_hi[:, 0:128])
            nc.sync.dma_start(out=vl_t[:, 0:128], in_=v_lo[:, 0:128])
            for sl in range(4):
                s0, s1 = sl * 1024, (sl + 1) * 1024
                nc.sync.dma_start(out=ah_t[:, s0:s1], in_=a_hi[:, s0:s1])
                nc.sync.dma_start(out=al_t[:, s0:s1], in_=a_lo[:, s0:s1])
            nc.sync.dma_start(out=iota_t[:, 0:2048], in_=iota_d[:, 0:2048])
            nc.sync.dma_start(out=vh_t[:, 128:256], in_=v_hi[:, 128:256])
            nc.sync.dma_start(out=vl_t[:, 128:256], in_=v_lo[:, 128:256])
            nc.sync.dma_start(out=iota_t[:, 2048:HW], in_=iota_d[:, 2048:HW])
            nc.sync.dma_start(out=vh_t[:, 256:JW], in_=v_hi[:, 256:JW])
            nc.sync.dma_start(out=vl_t[:, 256:JW], in_=v_lo[:, 256:JW])
            nc.sync.dma_start(out=w1tt_t[:], in_=w1tt[:])
            nc.sync.dma_start(out=w1vt_t[:], in_=w1vt[:])
            nc.sync.dma_start(out=b1_t[:], in_=b1d[:])
            nc.sync.dma_start(out=ident[:], in_=ident_d[:])
            nc.sync.dma_start(out=w2a_t[:], in_=w2ad[:])
            nc.sync.dma_start(out=w2b_t[:], in_=w2bd[:])
            nc.sync.dma_start(out=bna_t[:], in_=bnad[:])
            nc.sync.dma_start(out=bnb_t[:], in_=bnbd[:])

            fb3 = fused_b[:].rearrange("c (x y) -> c x y", y=YPAD)
            nc.gpsimd.memset(fused_b[:], 0.0)

            with tc.tile_pool(name="gdram", bufs=1, space="DRAM") as gdr:
                gt_dram = [gdr.tile([HW, C], F16, tag="gtdq", name="gtdq_t"),
                           gdr.tile([HW, C], F16, tag="gtdk", name="gtdk_t")]

                fa3 = fused_a[:].rearrange("c (x y) -> c x y", y=YPAD)
                nc.gpsimd.memset(fa3[:, :, 0:1], 0.0)
                nc.gpsimd.memset(fa3[:, :, YPAD - 1:YPAD], 0.0)
                nc.gpsimd.memset(fa3[:, 0:1, :], 0.0)
                # PSUM budget (8 banks): spine 2 tags x [128,1024] (4) +
                # conv1 cva/cvb (2) + conv2/G'-transpose c2a/c2b (2).
                with tc.tile_pool(name="sp_ps", bufs=1, space="PSUM") as sps, \
                     tc.tile_pool(name="sp_sb", bufs=2) as ssb, \
                     tc.tile_pool(name="sp_sm", bufs=4) as ssm, \
                     tc.tile_pool(name="sp_sc", bufs=4) as ssc, \
                     tc.tile_pool(name="cv_ps", bufs=1, space="PSUM") as cvp, \
                     tc.tile_pool(name="c2_ps", bufs=1, space="PSUM") as c2p, \
                     tc.tile_pool(name="s4_sb", bufs=2) as s4, \
                     tc.tile_pool(name="s4_dram", bufs=1, space="DRAM") as d4:

                    # ---- G' = W1t @ A + b1 (fp16), then transpose to DRAM.
                    g_sbs = [pp.tile([C, HW], F16, tag="gsbq", name="gsbq"),
                             pp.tile([C, HW], F16, tag="gsbk", name="gsbk")]

                    def gprime_piece(p):
                        for c8 in (2 * p, 2 * p + 1):
                            pms = [cvp.tile([C, 512], F32, tag="cva",
                                            name="gmq"),
                                   cvp.tile([C, 512], F32, tag="cvb",
                                            name="gmk")]
                            for m in range(2):
                                ro = m * C
                                nc.tensor.matmul(
                                    pms[m][:], w1tt_t[ro:ro + C, :],
                                    ah_t[ro:ro + C, c8 * 512:(c8 + 1) * 512],
                                    start=True, stop=True,
                                    tile_position=(ro, 0))
                            for m in range(2):
                                nc.scalar.activation(
                                    g_sbs[m][:, c8 * 512:(c8 + 1) * 512],
                                    pms[m][:],
                                    AF.Identity, bias=b1_t[:, 0:1],
                                    scale=1.0)
                        for m in range(2):
                            g_sb = g_sbs[m]
                            pst = c2p.tile([128, 512], F32,
                                           tag=f"c2{'ab'[p % 2]}",
                                           name="gtr")
                            stg = pp.tile([128, 512], F16, tag="stg")
                            for t in range(8):
                                blk = p * 8 + t
                                nc.tensor.matmul(
                                    pst[:, t * C:(t + 1) * C],
                                    g_sb[:, blk * 128:(blk + 1) * 128],
                                    ident[0:C, 0:C],
                                    start=True, stop=True)
                            nc.scalar.copy(stg[:], pst[:])
                            nc.sync.dma_start(
                                out=gt_dram[m][:]
                                .rearrange("(g p) c -> p g c", p=128)
                                [:, p * 8:(p + 1) * 8, :],
                                in_=stg[:].rearrange("p (g c) -> p g c", c=C))
                        if p == 3:
                            nc.scalar.copy(
                                fb3[:, 1:NSLOT, 1:H + 1],
                                vh_t[0:C, :]
                                .rearrange("c (x y) -> c x y", y=H))

                    def gather_jt(jt, m):
                        nc.gpsimd.indirect_dma_start(
                            out=gtile[m][:, jt * C:(jt + 1) * C],
                            out_offset=None,
                            in_=gt_dram[m][:],
                            in_offset=bass.IndirectOffsetOnAxis(
                                ap=idx_all[m][:, jt:jt + 1], axis=0),
                            bounds_check=HW - 1, oob_is_err=False)

                    def spine_mm(jt):
                        tsbs = [ssb.tile([128, HW], F32, tag="tsbq",
                                         name="tsbq"),
                                ssb.tile([128, HW], F32, tag="tsbk",
                                         name="tsbk")]
                        # 3-term fp16 score matmuls, 2-bank PSUM units.
                        # Tag alternates by unit parity so ACT evacuates one
                        # unit while PE fills the next; m is the outer loop
                        # so the q-row finishes (and its max pass starts)
                        # after 4 units instead of 7.
                        for m in range(2):
                            ro = m * C
                            for u in range(4):
                                psu = sps.tile([128, 1024], F32,
                                               tag=f"sp{u % 2}",
                                               name=f"sp{u % 2}")
                                for half in range(2):
                                    ch = 2 * u + half
                                    dst = psu[:, half * 512:(half + 1) * 512]
                                    i0 = ch * 512
                                    i1 = i0 + 512
                                    nc.tensor.matmul(
                                        dst, vh_t[ro:ro + C,
                                                  jt * 128:(jt + 1) * 128],
                                        ah_t[ro:ro + C, i0:i1],
                                        start=True, stop=False,
                                        tile_position=(ro, 0))
                                    nc.tensor.matmul(
                                        dst, vh_t[ro:ro + C,
                                                  jt * 128:(jt + 1) * 128],
                                        al_t[ro:ro + C, i0:i1],
                                        start=False, stop=False,
                                        tile_position=(ro, 0))
                                    nc.tensor.matmul(
                                        dst, vl_t[ro:ro + C,
                                                  jt * 128:(jt + 1) * 128],
                                        ah_t[ro:ro + C, i0:i1],
                                        start=False, stop=True,
                                        tile_position=(ro, 0))
                                nc.scalar.copy(
                                    tsbs[m][:, u * 1024:(u + 1) * 1024],
                                    psu[:])
                        return tsbs

                    def spine_max(jt, tsbs, m):
                        S = s_all[m][:, jt:jt + 1]
                        nc.vector.tensor_scalar(
                            out=tsbs[m][:], in0=tsbs[m][:],
                            scalar1=NEG, scalar2=NEG,
                            op0=OP.max, op1=OP.max, accum_out=S)

                    def spine_iseq(jt, tsbs, m, do_gather=True):
                        S = s_all[m][:, jt:jt + 1]
                        ist = ssm.tile([128, 1], F32, tag="ist",
                                       name="ist")
                        nc.vector.scalar_tensor_tensor(
                            out=tsbs[m][:], in0=tsbs[m][:], scalar=S,
                            in1=iota_t[:], op0=OP.is_equal,
                            op1=OP.mult, accum_out=ist[:])
                        nc.vector.tensor_copy(
                            idx_all[m][:, jt:jt + 1], ist[:])
                        if do_gather:
                            gather_jt(jt, m)
                            conv1_j(jt, m)

                    def spine_jt(jt, do_gather=True):
                        tsbs = spine_mm(jt)
                        for m in range(2):
                            spine_max(jt, tsbs, m)
                            spine_iseq(jt, tsbs, m, do_gather)

                    # conv1 in j-major: the gather already gives
                    # gtile[j, c]; the V 1x1 part lands [j, c] directly, the
                    # S-scale is an ACT per-partition scale (S is [j, 1]),
                    # and a PE transpose returns [c, j] for conv2.
                    c1_ps = {}

                    def conv1_j(jt, m):
                        g = jt // 4
                        if g not in c1_ps:
                            c1_ps[g] = (
                                cvp.tile([128, 512], F32, tag="cva",
                                         name="cva"),
                                cvp.tile([128, 512], F32, tag="cvb",
                                         name="cvb"))
                        ps1, ps2 = c1_ps[g]
                        i = jt - 4 * g
                        ro = m * C
                        dst = ps1[:, i * 128 + ro:i * 128 + ro + C]
                        nc.tensor.matmul(
                            dst, vh_t[ro:ro + C, jt * 128:(jt + 1) * 128],
                            w1vt_t[ro:ro + C, :],
                            start=True, stop=False, tile_position=(ro, 0))
                        nc.tensor.matmul(
                            dst, ident[:], gtile[m][:, jt * C:(jt + 1) * C],
                            start=False, stop=True)
                        scl = ssc.tile([128, C], F16, tag="scl", name="scl")
                        nc.scalar.activation(
                            scl[:], dst, AF.Identity, bias=0.0,
                            scale=s_all[m][:, jt:jt + 1])
                        nc.tensor.matmul(
                            ps2[ro:ro + C, i * 128:(i + 1) * 128],
                            scl[:], ident[:],
                            start=True, stop=True, tile_position=(0, ro))

                    def conv1_evac(g):
                        ps1, ps2 = c1_ps.pop(g)
                        jts = list(range(4 * g, min(4 * g + 4, 16)))
                        nt = len(jts)
                        x0 = 1 + jts[0] * 2
                        for m in range(2):
                            nc.scalar.copy(
                                fa3[m * C:(m + 1) * C,
                                    x0:x0 + 2 * nt, 1:H + 1],
                                ps2[m * C:(m + 1) * C, 0:nt * 128]
                                .rearrange("c (x y) -> c x y", y=H))

                    c2_tiles = {}

                    def conv1_halo():
                        ps1h = cvp.tile([128, 512], F32, tag="cva",
                                        name="cvah")
                        ps2h = cvp.tile([128, 512], F32, tag="cvb",
                                        name="cvbh")
                        for m in range(2):
                            ro = m * C
                            nc.tensor.matmul(
                                ps1h[ro:ro + C, 0:C],
                                vh_t[ro:ro + C, 2048:JW],
                                w1vt_t[ro:ro + C, :],
                                start=True, stop=False,
                                tile_position=(ro, ro))
                        for m in range(2):
                            nc.tensor.matmul(
                                ps1h[m * C:(m + 1) * C, 0:C],
                                ident[:, m * C:(m + 1) * C], gth[m][:],
                                start=False, stop=True,
                                tile_position=(0, m * C))
                        sclh = ssc.tile([128, C], F16, tag="scl",
                                        name="sclh")
                        nc.scalar.activation(
                            sclh[:], ps1h[:, 0:C], AF.Identity, bias=0.0,
                            scale=s_all[0][:, 16:17])
                        nc.tensor.matmul(
                            ps2h[0:C, 0:C], sclh[0:C, :], ident[0:C, 0:C],
                            start=True, stop=True, tile_position=(0, 0))
                        nc.tensor.matmul(
                            ps2h[C:128, 0:C], sclh[C:128, :],
                            ident[C:128, C:128],
                            start=True, stop=True, tile_position=(C, C))
                        nc.scalar.copy(
                            fa3[:, NSLOT - 1, 1:H + 1], ps2h[:, 0:C])

                    def spine_halo():
                        # Mixed tile: partitions 0-63 carry the q match's 64
                        # halo j-columns, 64-127 the k match's -- one DVE
                        # max/iseq pass instead of two half-empty tiles.
                        tsb = ssb.tile([128, HW], F32, tag="tsbq",
                                       name="tsbh")
                        for u in range(4):
                            psu = sps.tile([128, 1024], F32,
                                           tag=f"sp{u % 2}",
                                           name=f"sp{u % 2}")
                            for half in range(2):
                                ch = 2 * u + half
                                i0, i1 = ch * 512, ch * 512 + 512
                                for m in range(2):
                                    ro = m * C
                                    dst = psu[ro:ro + C,
                                              half * 512:(half + 1) * 512]
                                    nc.tensor.matmul(
                                        dst, vh_t[ro:ro + C, 2048:JW],
                                        ah_t[ro:ro + C, i0:i1],
                                        start=True, stop=False,
                                        tile_position=(ro, ro))
                                    nc.tensor.matmul(
                                        dst, vh_t[ro:ro + C, 2048:JW],
                                        al_t[ro:ro + C, i0:i1],
                                        start=False, stop=False,
                                        tile_position=(ro, ro))
                                    nc.tensor.matmul(
                                        dst, vl_t[ro:ro + C, 2048:JW],
                                        ah_t[ro:ro + C, i0:i1],
                                        start=False, stop=True,
                                        tile_position=(ro, ro))
                            nc.scalar.copy(
                                tsb[:, u * 1024:(u + 1) * 1024], psu[:])
                        S = s_all[0][:, 16:17]
                        nc.vector.tensor_scalar(
                            out=tsb[:], in0=tsb[:],
                            scalar1=NEG, scalar2=NEG,
                            op0=OP.max, op1=OP.max, accum_out=S)
                        ist = ssm.tile([128, 1], F32, tag="ist", name="ist")
                        nc.vector.scalar_tensor_tensor(
                            out=tsb[:], in0=tsb[:], scalar=S,
                            in1=iota_t[:], op0=OP.is_equal,
                            op1=OP.mult, accum_out=ist[:])
                        nc.vector.tensor_copy(
                            idx_all[0][:, 16:17], ist[:])
                        # full-partition gathers (partition-offset indirect
                        # DMA is not supported); rows 0-63 of gth[0] and
                        # 64-127 of gth[1] are the meaningful halves, and the
                        # inject matmuls select them via identity columns.
                        for m in range(2):
                            nc.gpsimd.indirect_dma_start(
                                out=gth[m][:],
                                out_offset=None,
                                in_=gt_dram[m][:],
                                in_offset=bass.IndirectOffsetOnAxis(
                                    ap=idx_all[0][:, 16:17], axis=0),
                                bounds_check=HW - 1, oob_is_err=False)
                        conv1_halo()

                    def conv2_piece(oc, half, q0, q1):
                        # output rows [oc*16 + half*8 + q0, ... + q1) of y;
                        # pieces of the same (oc, half) share one PSUM bank.
                        key = (oc, half)
                        if key not in c2_tiles:
                            c2_tiles[key] = c2p.tile(
                                [128, 512], F32, tag=f"c2{'ab'[half]}",
                                name=f"c2{'ab'[half]}")
                        psm = c2_tiles[key]
                        n0, n1 = q0 * H, q1 * H
                        for t in range(9):
                            dx, dy = t // 3, t % 3
                            ox = 1 + oc * 16 + half * 8 + q0
                            ra = fa3[:, ox + dx - 1:ox + dx - 1 + q1 - q0,
                                     dy:dy + H]
                            rb = fb3[:, ox + dx - 1:ox + dx - 1 + q1 - q0,
                                     dy:dy + H]
                            nc.tensor.matmul(
                                psm[half * C:(half + 1) * C, n0:n1],
                                w2a_t[:, t * C:(t + 1) * C], ra,
                                start=(t == 0), stop=False,
                                tile_position=(0, half * C))
                            nc.tensor.matmul(
                                psm[half * C:(half + 1) * C, n0:n1],
                                w2b_t[:, t * C:(t + 1) * C], rb,
                                start=False, stop=(t == 8),
                                tile_position=(0, half * C))
                        nc.scalar.activation(
                            out_sb[half * C:(half + 1) * C,
                                   oc * 512 + n0:oc * 512 + n1],
                            psm[half * C:(half + 1) * C, n0:n1],
                            AF.Relu,
                            bias=bnb_t[half * C:(half + 1) * C, 0:1],
                            scale=bna_t[half * C:(half + 1) * C, 0:1])
                        y3 = yout[:].rearrange("c (x y) -> c x y", y=H)
                        r0 = oc * 16 + half * 8 + q0
                        nc.sync.dma_start(
                            out=y3[:, r0:r0 + q1 - q0, :],
                            in_=out_sb[half * C:(half + 1) * C,
                                       oc * 512 + n0:oc * 512 + n1]
                            .rearrange("c (x y) -> c x y", y=H))

                    def conv2_chunk(oc, halves=(0, 1)):
                        for half in halves:
                            conv2_piece(oc, half, 0, 8)

                    # Prelude: two spine j-tiles so DVE/ACT fill up before
                    # the PE runs the G' phase; then steady-state groups.
                    # conv1's S-multiply for group g-1 is emitted after the
                    # first j-tile of group g so the s_bc DRAM round trip
                    # never blocks the DVE queue head.
                    # Startup: the G' phase is interleaved piecewise with
                    # the first four spine j-tiles so its ACT work (bias +
                    # transpose staging) never forms a solid window that
                    # delays spine PSUM evacuations.  Gathers for those
                    # tiles are deferred until gt_dram is complete (the
                    # indirect-DMA source is not dependency-tracked, so
                    # program order must enforce the G' -> gather ordering).
                    for jt in range(4):
                        spine_jt(jt, do_gather=False)
                        gprime_piece(jt)
                    for jt in range(4):
                        for m in range(2):
                            gather_jt(jt, m)
                            conv1_j(jt, m)
                    for g in range(4):
                        jts = list(range(4 * g, 4 * g + 4))
                        for jt in jts:
                            if g > 0:
                                spine_jt(jt)
                        conv1_evac(g)
                        if g == 3:
                            conv2_chunk(0)
                            conv2_chunk(1, halves=(0,))
                            conv2_piece(1, 1, 0, 4)
                            conv2_piece(1, 1, 4, 6)
                    spine_halo()
                    conv2_piece(1, 1, 6, 8)

    nc.finalize()
    return nc


_NC_CACHE = None


def _get_nc():
    global _NC_CACHE
    if _NC_CACHE is None:
        _NC_CACHE = _build_nc()
    return _NC_CACHE


def _host_prep(inputs):
    V = np.ascontiguousarray(inputs["V_rgb"], dtype=np.float32)
    K = np.ascontiguousarray(inputs["K_dep"], dtype=np.float32)
    Q = np.ascontiguousarray(inputs["Q_flo"], dtype=np.float32)
    w1 = np.asarray(inputs["conv1_w"], dtype=np.float32)[:, :, 0, 0]
    b1 = np.asarray(inputs["conv1_b"], dtype=np.float32)
    w2 = np.asarray(inputs["conv2_w"], dtype=np.float32)
    b2 = np.asarray(inputs["conv2_b"], dtype=np.float32)
    g = np.asarray(inputs["bn_gamma"], dtype=np.float32)
    be = np.asarray(inputs["bn_beta"], dtype=np.float32)
    mu = np.asarray(inputs["bn_mean"], dtype=np.float32)
    var = np.asarray(inputs["bn_var"], dtype=np.float32)

    w1vt1 = w1[:, :C].T.astype(np.float16)
    w1vt = np.ascontiguousarray(np.concatenate([w1vt1, w1vt1], axis=0))
    w1tt1 = w1[:, C:].T.astype(np.float16)
    w1tt = np.ascontiguousarray(np.concatenate([w1tt1, w1tt1], axis=0))
    # Odd cores store their window x-flipped (edge-to-interior order), so
    # their conv2 kernel is x-flipped to match.
    def build_w2(flip):
        w2a = np.zeros((128, 9 * C), np.float16)
        w2b = np.zeros((C, 9 * C), np.float16)
        for t in range(9):
            dx, dy = t // 3, t % 3
            sdx = 2 - dx if flip else dx
            lhsT = w2[:, :, sdx, dy].T.astype(np.float16)  # [192, 64]
            w2a[:, t * C:(t + 1) * C] = lhsT[0:128]
            w2b[:, t * C:(t + 1) * C] = lhsT[128:192]
        return w2a, w2b

    w2ab = [build_w2(False), build_w2(True)]
    bna = g / np.sqrt(var + BN_EPS)
    bnb = be + (b2 - mu) * bna
    bna2 = np.ascontiguousarray(np.concatenate([bna, bna])[:, None])
    bnb2 = np.ascontiguousarray(np.concatenate([bnb, bnb])[:, None])

    def split16(x):
        hi = x.astype(np.float16)
        lo = (x - hi.astype(np.float32)).astype(np.float16)
        return hi, lo

    in_maps = []
    for core in range(N_CORES):
        b, half = core // 2, core % 2
        # stored rows: 32 own rows ordered sample-edge -> interior, then the
        # one real (interior-side) halo row.  Odd cores are x-flipped.
        if half == 0:
            rows = list(range(0, 32)) + [32]
        else:
            rows = list(range(63, 31, -1)) + [31]
        vw = np.ascontiguousarray(
            V[b][:, rows, :].astype(np.float32)).reshape(C, JW)
        a2 = np.concatenate(
            [Q[b].reshape(C, HW), K[b].reshape(C, HW)], axis=0)
        a_hi, a_lo = split16(a2)
        vwh, vwl = split16(vw)
        v_hi = np.concatenate([vwh, vwh], axis=0)
        v_lo = np.concatenate([vwl, vwl], axis=0)
        in_maps.append({
            "a_hi": np.ascontiguousarray(a_hi),
            "a_lo": np.ascontiguousarray(a_lo),
            "v_hi": np.ascontiguousarray(v_hi),
            "v_lo": np.ascontiguousarray(v_lo),
            "w1vt": w1vt,
            "w1tt": w1tt,
            "b1d": np.ascontiguousarray(b1[:, None]),
            "w2ad": w2ab[half][0],
            "w2bd": w2ab[half][1],
            "bnad": bna2,
            "bnbd": bnb2,
        })
    return in_maps


def kernel(**inputs):
    nc = _get_nc()
    in_maps = _host_prep(inputs)
    res = bass_utils.run_bass_kernel_spmd(
        nc, in_maps, core_ids=list(range(N_CORES)))
    y = np.zeros((B, C, W, H), np.float32)
    for core in range(N_CORES):
        b, half = core // 2, core % 2
        out = res.results[core]["y"].reshape(C, OUT_ROWS, H)
        if half == 0:
            y[b, :, 0:OUT_ROWS, :] = out
        else:
            y[b, :, OUT_ROWS:W, :] = out[:, ::-1, :]
    return y
